# revision 11
# baseline (speedup 1.0000x reference)
"""Trainium2 Bass kernel for nn_Interpolator (ragged sequence interpolation).

Reference computation (N=32768 obs, R=2048 ref timesteps, ninp=64):
    d2[r,n]   = (ref[r] - t[n])^2
    Ks        = exp(-a*d2)*mask + EPS        (mask = t>0)
    Kc        = exp(-10a*d2)*mask + EPS
    lam_s     = Ks @ onehot(dims) + EPS      [R,64]
    num_s     = Ks @ (onehot*v)              [R,64]
    (same for coarse kernel Kc)
    lam       = lam_s / R
    cross     = (num_s @ rho) / rowsum(lam_s)     (1/R cancels)
    coarse    = num_c / lam_c
    transient = coarse - cross
    out       = concat([lam, cross, transient], -1)   [1, R, 192]

Strategy: shard the observation axis N across 8 cores.  Each core computes
its [128, R] kernel slabs fully on-chip (SBUF; the [R,N] matrices never
touch HBM), accumulates per-dimension segment sums via PE matmuls with
one-hot stationary weights (col-tiled: onehot in array cols 0:63, v*onehot
in 64:127 so lam and num come out of one streaming pass into one PSUM
bank), AllReduces the [2,128,R] partials, and every core (replicated)
finishes the tiny per-R math + transposes + writes the output.
"""

import os
import sys

import numpy as np

sys.path.insert(0, "/opt/trn_rl_repo")

import concourse.bass as bass
import concourse.tile as tile
from concourse import bacc, mybir
from concourse.masks import make_identity

# The image's antenv package lacks axon_hooks (NTFF profiling registry);
# register one so trace=True can profile HW exec time. Harmless if unused.
try:
    import antenv.axon_hooks  # noqa: F401
except ImportError:
    import importlib.util as _ilu
    import types as _types

    _m = _types.ModuleType("antenv.axon_hooks")
    _m._hook = None

    def _set_hook(hook):
        _m._hook = hook

    def _get_hook():
        if _m._hook is None:
            try:
                from trn_agent_boot.trn_boot import _ntff_profile_via_ctypes

                _m._hook = _ntff_profile_via_ctypes("/opt/axon/libaxon_pjrt.so")
            except Exception:
                _m._hook = None
        return _m._hook

    _m.set_axon_ntff_profile_hook = _set_hook
    _m.get_axon_ntff_profile_hook = _get_hook
    sys.modules["antenv.axon_hooks"] = _m
    try:
        import antenv

        antenv.axon_hooks = _m
    except ImportError:
        pass

F32 = mybir.dt.float32
Alu = mybir.AluOpType
Act = mybir.ActivationFunctionType

# Problem constants (hardcoded; kernel.py must be self-contained).
N = 32768
R = 2048
NI = 64          # ninp
M = 8            # cores
ND = N // M      # 4096 obs per core
P = 128          # partition dim / chunk size
NCHUNK = ND // P # 32
RB = 512         # psum bank width (fp32)
NRB = R // RB    # 4
EPS = 1e-7
K_SCALE = 10.0


def build_program(alpha: float):
    """Build the SPMD bass program (same program on all 8 cores)."""
    # Bacc (not raw Bass): its generate_event_semaphores pass splits
    # multi-sem waits into EventSemaphore insts — walrus allows only one
    # sync wait per compute instruction.
    nc = bacc.Bacc("TRN2")

    s_in = nc.declare_dram_parameter("s", [ND, 3], F32, isOutput=False)
    ref_in = nc.declare_dram_parameter("ref", [R], F32, isOutput=False)
    rho_in = nc.declare_dram_parameter("rho", [NI, NI], F32, isOutput=False)
    # corr[0:64]  = EPS*(cnt_k+1)  (lam correction, real values on core 0 only)
    # corr[64:128]= EPS*sv_k       (num correction)
    corr_in = nc.declare_dram_parameter("corr", [P, 1], F32, isOutput=False)
    out_t = nc.declare_dram_parameter("out", [R, 3 * NI], F32, isOutput=True)

    with tile.TileContext(nc) as tc:
        with (
            tc.tile_pool(name="consts", bufs=1) as consts,
            tc.tile_pool(name="dram", bufs=1, space="DRAM") as dram,
        ):
            # ---------------- constants ----------------
            sdata = consts.tile([P, NCHUNK, 3], F32)
            nc.sync.dma_start(
                out=sdata[:], in_=s_in[:].rearrange("(c p) k -> p c k", p=P)
            )
            refrow = consts.tile([1, R], F32)
            nc.sync.dma_start(out=refrow[:], in_=ref_in[None, :])
            corr_col = consts.tile([P, 1], F32)
            nc.sync.dma_start(out=corr_col[:], in_=corr_in[:])
            rho_sb = consts.tile([NI, NI], F32)
            nc.sync.dma_start(out=rho_sb[:], in_=rho_in[:])

            ones_row = consts.tile([1, P], F32)
            nc.vector.memset(ones_row, 1.0)
            ones_col = consts.tile([NI, 1], F32)
            nc.vector.memset(ones_col, 1.0)
            # walrus only allows ONE sync wait on a Matmult (it lands on the
            # LDWEIGHTS micro-op).  Every matmul below therefore keeps both
            # operands' producers on a single engine: DVE-copied constants
            # (refrow2/ident2/rho2/...) or ACT-copied weights (combA).
            identity = consts.tile([P, P], F32)
            make_identity(nc, identity)
            ident2 = consts.tile([P, P], F32)
            nc.vector.tensor_copy(out=ident2[:], in_=identity[:])
            refrow2 = consts.tile([1, R], F32)
            nc.vector.tensor_copy(out=refrow2[:], in_=refrow[:])

            iota_i = consts.tile([P, NI], mybir.dt.int32)
            nc.gpsimd.iota(iota_i, pattern=[[1, NI]], channel_multiplier=0)
            iota_f = consts.tile([P, NI], F32)
            nc.vector.tensor_copy(out=iota_f, in_=iota_i)

            # ref broadcast to all 128 partitions via PE outer product
            ref_bcast = consts.tile([P, R], F32)
            with tc.tile_pool(name="bps", bufs=2, space="PSUM") as bps:
                for b in range(NRB):
                    pb = bps.tile([P, RB], F32, tag="pb")
                    nc.tensor.matmul(
                        pb[:],
                        ones_row[0:1, :],
                        refrow2[0:1, b * RB : (b + 1) * RB],
                        start=True,
                        stop=True,
                    )
                    nc.scalar.copy(out=ref_bcast[:, b * RB : (b + 1) * RB], in_=pb[:])

            part = consts.tile([P, 2, R], F32)  # [:,0,:]=smooth, [:,1,:]=coarse

            # ---------------- main loop ----------------
            with (
                tc.tile_pool(name="acc", bufs=1, space="PSUM") as accpool,
                tc.tile_pool(name="work", bufs=3) as work,
                tc.tile_pool(name="kmat", bufs=2) as kmat,
            ):
                accs = {}
                for qi in range(2):
                    for rb in range(NRB):
                        accs[qi, rb] = accpool.tile(
                            [P, RB], F32, name=f"acc_{qi}_{rb}", tag=f"acc_{qi}_{rb}"
                        )

                for c in range(NCHUNK):
                    t_c = sdata[:, c, 0:1]
                    v_c = sdata[:, c, 1:2]
                    d_c = sdata[:, c, 2:3]

                    mask = work.tile([P, 1], F32, tag="mask")
                    nc.vector.tensor_scalar(
                        out=mask[:], in0=t_c, scalar1=0.0, scalar2=None, op0=Alu.is_gt
                    )
                    comb = work.tile([P, 2 * NI], F32, tag="comb")
                    nc.vector.tensor_scalar(
                        out=comb[:, 0:NI],
                        in0=iota_f[:],
                        scalar1=d_c,
                        scalar2=mask[:],
                        op0=Alu.is_equal,
                        op1=Alu.mult,
                    )
                    nc.vector.tensor_scalar(
                        out=comb[:, NI : 2 * NI],
                        in0=comb[:, 0:NI],
                        scalar1=v_c,
                        scalar2=None,
                        op0=Alu.mult,
                    )

                    combA = work.tile([P, 2 * NI], F32, tag="combA")
                    nc.scalar.copy(out=combA[:], in_=comb[:])

                    diff = work.tile([P, R], F32, tag="diff")
                    nc.vector.tensor_scalar(
                        out=diff[:],
                        in0=ref_bcast[:],
                        scalar1=t_c,
                        scalar2=None,
                        op0=Alu.subtract,
                    )
                    d2 = work.tile([P, R], F32, tag="d2")
                    nc.vector.tensor_mul(out=d2[:], in0=diff[:], in1=diff[:])

                    ks = kmat.tile([P, R], F32, tag="ks")
                    nc.scalar.activation(out=ks[:], in_=d2[:], func=Act.Exp,
                                         scale=-alpha)
                    kc = kmat.tile([P, R], F32, tag="kc")
                    nc.scalar.activation(out=kc[:], in_=d2[:], func=Act.Exp,
                                         scale=-alpha * K_SCALE)

                    for qi, kk in ((0, ks), (1, kc)):
                        for rb in range(NRB):
                            acc = accs[qi, rb]
                            blk = kk[:, rb * RB : (rb + 1) * RB]
                            nc.tensor.matmul(
                                acc[0:NI, :], combA[:, 0:NI], blk,
                                start=(c == 0), stop=(c == NCHUNK - 1),
                                skip_group_check=True,
                            )
                            nc.tensor.matmul(
                                acc[NI:P, :], combA[:, NI : 2 * NI], blk,
                                start=(c == 0), stop=(c == NCHUNK - 1),
                                skip_group_check=True,
                            )

                # drain psum -> sbuf, adding the EPS corrections (core 0 only
                # carries nonzero corr; the AllReduce applies it once globally)
                for qi in range(2):
                    for rb in range(NRB):
                        nc.vector.tensor_scalar(
                            out=part[:, qi, rb * RB : (rb + 1) * RB],
                            in0=accs[qi, rb][:],
                            scalar1=corr_col[:],
                            scalar2=None,
                            op0=Alu.add,
                        )

            # ---------------- all-reduce partials ----------------
            ar_in = dram.tile([P, 2, R], F32, name="ar_in")
            ar_out = dram.tile([P, 2, R], F32, name="ar_out", addr_space="Shared")
            nc.sync.dma_start(out=ar_in[:], in_=part[:])
            nc.gpsimd.collective_compute(
                "AllReduce",
                Alu.add,
                replica_groups=[list(range(M))],
                ins=[ar_in[:].opt()],
                outs=[ar_out[:].opt()],
            )
            ls_t = consts.tile([NI, R], F32)   # lam_s
            ns_t = consts.tile([NI, R], F32)   # num_s
            lc_t = consts.tile([NI, R], F32)   # lam_c
            nc_t = consts.tile([NI, R], F32)   # num_c
            nc.sync.dma_start(out=ls_t[:], in_=ar_out[0:NI, 0, :])
            nc.sync.dma_start(out=ns_t[:], in_=ar_out[NI:P, 0, :])
            nc.sync.dma_start(out=lc_t[:], in_=ar_out[0:NI, 1, :])
            nc.sync.dma_start(out=nc_t[:], in_=ar_out[NI:P, 1, :])
            ls = ls_t[:]
            ns = ns_t[:]
            lc = lc_t[:]
            ncc = nc_t[:]

            # ---------------- finishing (replicated) ----------------
            with tc.tile_pool(name="fin", bufs=1) as fin:
              with tc.tile_pool(name="fps", bufs=2, space="PSUM") as fps:
                ls2 = fin.tile([NI, R], F32)
                nc.vector.tensor_copy(out=ls2[:], in_=ls)
                ns2 = fin.tile([NI, R], F32)
                nc.vector.tensor_copy(out=ns2[:], in_=ns)
                rho2 = fin.tile([NI, NI], F32)
                nc.vector.tensor_copy(out=rho2[:], in_=rho_sb[:])

                lam_out = fin.tile([NI, R], F32)
                nc.vector.tensor_scalar(
                    out=lam_out[:], in0=ls, scalar1=1.0 / R, scalar2=None, op0=Alu.mult
                )
                rec_lc = fin.tile([NI, R], F32)
                nc.vector.reciprocal(out=rec_lc[:], in_=lc)
                coarse = fin.tile([NI, R], F32)
                nc.vector.tensor_mul(out=coarse[:], in0=ncc, in1=rec_lc[:])

                # D[r] = sum_k lam_s[k, r]; recD = 1/D
                recd = fin.tile([1, R], F32)
                for b in range(NRB):
                    dps = fps.tile([1, RB], F32, tag="dps")
                    nc.tensor.matmul(
                        dps[:], ones_col[:], ls2[:, b * RB : (b + 1) * RB],
                        start=True, stop=True,
                    )
                    nc.vector.reciprocal(
                        out=recd[:, b * RB : (b + 1) * RB], in_=dps[:]
                    )

                # cross = (rho^T-contract num_s) * recD  (broadcast over k)
                cross = fin.tile([NI, R], F32)
                dbc = fin.tile([NI, R], F32)
                for b in range(NRB):
                    crp = fps.tile([NI, RB], F32, tag="crp")
                    nc.tensor.matmul(
                        crp[:], rho2[:], ns2[:, b * RB : (b + 1) * RB],
                        start=True, stop=True,
                    )
                    dbp = fps.tile([NI, RB], F32, tag="dbp")
                    nc.tensor.matmul(
                        dbp[:], ones_row[0:1, 0:NI],
                        recd[0:1, b * RB : (b + 1) * RB],
                        start=True, stop=True,
                    )
                    nc.scalar.copy(
                        out=dbc[:, b * RB : (b + 1) * RB], in_=dbp[:]
                    )
                    nc.vector.tensor_mul(
                        out=cross[:, b * RB : (b + 1) * RB],
                        in0=crp[:],
                        in1=dbc[:, b * RB : (b + 1) * RB],
                    )

                transient = fin.tile([NI, R], F32)
                nc.vector.tensor_sub(out=transient[:], in0=coarse[:], in1=cross[:])

              # transpose [64, R] slabs to [R, 192] output rows
              with (
                    tc.tile_pool(name="outp", bufs=3) as outp,
                    tc.tile_pool(name="tps", bufs=4, space="PSUM") as tps,
              ):
                    for rb16 in range(R // P):
                        ot = outp.tile([P, 3 * NI], F32, tag="ot")
                        for slot, src in enumerate((lam_out, cross, transient)):
                            tp = tps.tile([P, NI], F32, tag="tp")
                            nc.tensor.transpose(
                                tp[:],
                                src[:, rb16 * P : (rb16 + 1) * P],
                                ident2[0:NI, 0:NI],
                            )
                            nc.vector.tensor_copy(
                                out=ot[:, slot * NI : (slot + 1) * NI], in_=tp[:]
                            )
                        nc.sync.dma_start(
                            out=out_t[rb16 * P : (rb16 + 1) * P, :], in_=ot[:]
                        )

    nc.finalize()
    return nc


_prog_cache = {}


def _get_prog(alpha: float):
    key = round(float(alpha), 9)
    if key not in _prog_cache:
        _prog_cache[key] = build_program(float(alpha))
    return _prog_cache[key]


last_results = None  # BassKernelResults of the most recent run (for test.py)


def kernel(S, reference_timesteps, alpha, rho):
    global last_results
    S = np.ascontiguousarray(np.asarray(S, dtype=np.float32))
    ref = np.ascontiguousarray(np.asarray(reference_timesteps, dtype=np.float32))
    rho = np.ascontiguousarray(np.asarray(rho, dtype=np.float32))
    a = float(np.asarray(alpha).reshape(-1)[0])

    assert S.shape == (N, 3) and ref.shape == (1, R) and rho.shape == (NI, NI)

    nc = _get_prog(a)

    # host-side EPS-correction constants (O(N) prep, applied once via core 0)
    dims = S[:, 2].astype(np.int32)
    v = S[:, 1].astype(np.float64)
    cnt = np.bincount(dims, minlength=NI).astype(np.float64)
    sv = np.bincount(dims, weights=v, minlength=NI)
    corr = np.concatenate([EPS * (cnt + 1.0), EPS * sv]).astype(np.float32)
    corr = corr.reshape(P, 1)
    zcorr = np.zeros((P, 1), np.float32)

    in_maps = []
    for i in range(M):
        in_maps.append(
            {
                "s": S[i * ND : (i + 1) * ND],
                "ref": ref[0],
                "rho": rho,
                "corr": corr if i == 0 else zcorr,
            }
        )

    if os.environ.get("BASS_SIM"):
        from concourse.bass_interp import MultiCoreSim

        sim = MultiCoreSim(nc, M)
        for i in range(M):
            for k, val in in_maps[i].items():
                sim.cores[i].tensor(k)[:] = val
        sim.simulate()
        out = np.array(sim.cores[0].tensor("out"))
        last_results = None
    else:
        from concourse.bass_utils import run_bass_kernel_spmd

        res = run_bass_kernel_spmd(
            nc,
            in_maps,
            list(range(M)),
            trace=bool(os.environ.get("BASS_TRACE")),
        )
        last_results = res
        out = np.asarray(res.results[0]["out"])

    return out.reshape(1, R, 3 * NI).astype(np.float32)
